# revision 9
# baseline (speedup 1.0000x reference)
"""Trainium2 Bass kernel for IPTConv: pixel-unshuffle(scale=2, channel-major)
followed by a 1x1 pointwise conv (GEMM) + bias.

    y[b, o, h', w'] = sum_{c,s1,s2} W[o, c*4 + 2*s1 + s2] * x[b, c, 2h'+s1, 2w'+s2] + bias[o]

Full shapes: x [16, 64, 256, 256] f32, W [128, 256] f32, b [128] f32
             y [16, 128, 128, 128] f32

Sharding: data-parallel over batch — 2 batches per core on 8 cores, weights
replicated, no cross-core communication.

Per-core kernel layout:
  - x tile in SBUF: [128 partitions, NH, 256] where partition p = s1*64 + c
    holds x[b, c, 2h'+s1, :] rows (even h rows on partitions 0-63, odd on
    64-127). Loaded with 2 HWDGE DMAs (1 MiB each, contiguous 1KB runs).
  - matmul: out[o, (h',w')] += wt_s2[(s1,c), o].T @ x_view[(s1,c), (h', 2w'+s2)]
    with K=128, M=128, N=512, accumulating s2=0,1 into one PSUM bank.
  - PSUM -> SBUF evacuation fused with bias add on the vector engine.
  - output DMA: [128, NH, 128] -> y[b, :, h'block, :]  (1 MiB per DMA).

Weights are pre-permuted on the host (128x256 f32 = 128KB, negligible) into
the two stationary lhsT tiles wt_s2[p, o] = W[o, (p%64)*4 + (p//64)*2 + s2].
"""

import numpy as np

import concourse.bass as bass
import concourse.bacc as bacc
import concourse.mybir as mybir
import concourse.tile as tile
from concourse.bass_utils import run_bass_kernel_spmd

N_CORES = 8
B_FULL, C1, H, WD = 16, 64, 256, 256
C2 = 128
S = 2
BPC = B_FULL // N_CORES          # batches per core
HP, WP = H // S, WD // S         # 128, 128 output spatial
NH = 16                          # output h' rows per x tile
HB = HP // NH                    # h' blocks per batch
SUBH = 4                         # h' rows per PSUM tile (SUBH*WP = 512 = bank)
NSUB = NH // SUBH

_FP = mybir.dt.float32


def build_nc(mm_dtype=_FP, bpc=BPC, hb_blocks=HB):
    nc = bacc.Bacc("TRN2", target_bir_lowering=False, debug=False)
    xs = nc.declare_dram_parameter("xs", [bpc, C1, 2 * NH * hb_blocks, WD], _FP,
                                   isOutput=False)
    wt0 = nc.declare_dram_parameter("wt0", [128, C2], _FP, isOutput=False)
    wt1 = nc.declare_dram_parameter("wt1", [128, C2], _FP, isOutput=False)
    bias = nc.declare_dram_parameter("bias", [C2, 1], _FP, isOutput=False)
    y = nc.declare_dram_parameter("y", [bpc, C2, NH * hb_blocks, WP], _FP,
                                  isOutput=True)

    with tile.TileContext(nc) as tc:
        with (
            tc.tile_pool(name="consts", bufs=1) as consts,
            tc.tile_pool(name="xp", bufs=3) as xp,
            tc.tile_pool(name="op", bufs=3) as op,
            tc.tile_pool(name="psp", bufs=6, space="PSUM") as psp,
        ):
            wt0_sb = consts.tile([128, C2], _FP)
            wt1_sb = consts.tile([128, C2], _FP)
            bias_sb = consts.tile([C2, 1], _FP)
            nc.sync.dma_start(out=wt0_sb, in_=wt0[:, :])
            nc.sync.dma_start(out=wt1_sb, in_=wt1[:, :])
            nc.sync.dma_start(out=bias_sb, in_=bias[:, :])
            wt_mm = [
                wt0_sb[:, :].bitcast(mm_dtype),
                wt1_sb[:, :].bitcast(mm_dtype),
            ]
            # All-engine barrier after the weight preamble: matmuls have a
            # hard cap on attached semaphore waits (fp32 matmul self-loads
            # weights, so waits can't be moved to a standalone ldweights) —
            # don't let the first matmul inherit the 3 weight-DMA waits.
            tc.strict_bb_all_engine_barrier()

            # [b, c, h, w] -> [b, s1, c, hp, w] with h = 2*hp + s1
            xv = xs.rearrange("b c (hp s1) w -> b s1 c hp w", s1=S)

            for b in range(bpc):
                for hb in range(hb_blocks):
                    hsl = slice(hb * NH, (hb + 1) * NH)
                    x2 = xp.tile([128, NH, WD], _FP)
                    # DMA APs allow at most 3 dims, so the (s1, c) partition
                    # split needs one DMA per h-parity: even h rows land on
                    # partitions 0-63, odd on 64-127 (disjoint SDMA engines).
                    nc.sync.dma_start(out=x2[0:64], in_=xv[b, 0, :, hsl, :])
                    nc.sync.dma_start(out=x2[64:128], in_=xv[b, 1, :, hsl, :])
                    # [p, hp, w] -> [p, hp, wp, s2] with w = 2*wp + s2
                    x2v = x2[:, :, :].rearrange("p hp (wp s2) -> p hp wp s2", s2=S)
                    ot = op.tile([C2, NH, WP], _FP)
                    for sub in range(NSUB):
                        ssl = slice(sub * SUBH, (sub + 1) * SUBH)
                        ps = psp.tile([C2, SUBH, WP], _FP)
                        for s2 in range(S):
                            rhs = x2v[:, ssl, :, s2].bitcast(mm_dtype)
                            nc.tensor.matmul(
                                ps[:, :, :], wt_mm[s2], rhs,
                                start=(s2 == 0), stop=(s2 == 1),
                            )
                        nc.vector.tensor_scalar_add(
                            out=ot[:, ssl, :], in0=ps[:, :, :],
                            scalar1=bias_sb[:, :],
                        )
                    nc.sync.dma_start(out=y[b, :, hsl, :], in_=ot)
    nc.compile()
    return nc


_F32R = mybir.dt.float32r


def build_nc2(mm_dtype=_F32R, hb_blocks=HB):
    """v2: batch-pair partition layout. Partition p = q*64 + c (q = batch),
    so every partition's DMA read is one contiguous 32KB run (single 4MB
    DMA per block at near-peak HBM bw). Contraction drops to K=64 with all
    four (s1, s2) taps as separate accumulating matmuls; the q=0/q=1 matmul
    streams use PE row-groups 0-63 / 64-127 concurrently (tile_position
    auto-derived from base_partition). float32r (tf32) matmuls run at
    1 cyc/row vs fp32's 4, keeping PE off the critical path."""
    nc = bacc.Bacc("TRN2", target_bir_lowering=False, debug=False)
    NH2 = 2 * NH  # 32 h rows per block, 16 output rows
    xs = nc.declare_dram_parameter("xs", [BPC, C1, NH2 * hb_blocks, WD],
                                   mm_dtype, isOutput=False)
    wts = nc.declare_dram_parameter("wts", [128, 4, C2], mm_dtype,
                                    isOutput=False)
    bias = nc.declare_dram_parameter("bias", [C2, 1], _FP, isOutput=False)
    y = nc.declare_dram_parameter("y", [BPC, C2, NH * hb_blocks, WP], _FP,
                                  isOutput=True)

    with tile.TileContext(nc) as tc:
        with (
            tc.tile_pool(name="consts", bufs=1) as consts,
            tc.tile_pool(name="xp", bufs=2) as xp,
            tc.tile_pool(name="op", bufs=4) as op,
            tc.tile_pool(name="psp", bufs=8, space="PSUM") as psp,
        ):
            wts_sb = consts.tile([128, 4, C2], mm_dtype)
            bias_sb = consts.tile([C2, 1], _FP)
            nc.sync.dma_start(out=wts_sb, in_=wts[:, :, :])
            nc.sync.dma_start(out=bias_sb, in_=bias[:, :])
            wts_mm = wts_sb[:, :, :]
            tc.strict_bb_all_engine_barrier()

            for hb in range(hb_blocks):
                hsl2 = slice(hb * NH2, (hb + 1) * NH2)
                hsl = slice(hb * NH, (hb + 1) * NH)
                x2 = xp.tile([128, NH2, WD], mm_dtype)
                # (b, c, 32h, 256w) -> partitions (b*64+c), contiguous free
                nc.sync.dma_start(out=x2[:, :, :], in_=xs[:, :, hsl2, :])
                # [p, (hp s1), (wp s2)] view
                x2r = x2[:, :, :].rearrange(
                    "p (hp s1) (wp s2) -> p hp s1 wp s2", s1=S, s2=S)
                for q in range(BPC):
                    qsl = slice(q * 64, (q + 1) * 64)
                    ot = op.tile([C2, NH, WP], _FP)
                    for sub in range(NSUB):
                        ssl = slice(sub * SUBH, (sub + 1) * SUBH)
                        ps = psp.tile([C2, SUBH, WP], _FP)
                        for s1 in range(S):
                            for s2 in range(S):
                                rhs = x2r[qsl, ssl, s1, :, s2]
                                nc.tensor.matmul(
                                    ps[:, :, :],
                                    wts_mm[qsl, s1 * 2 + s2, :],
                                    rhs,
                                    start=(s1 == 0 and s2 == 0),
                                    stop=(s1 == 1 and s2 == 1),
                                )
                        nc.vector.tensor_scalar_add(
                            out=ot[:, ssl, :], in0=ps[:, :, :],
                            scalar1=bias_sb[:, :],
                        )
                    nc.sync.dma_start(out=y[q, :, hsl, :], in_=ot)
    nc.compile()
    return nc


def make_weight_inputs2(W, b):
    # wts[p, j, o] = W[o, (p%64)*4 + j], duplicated across partition halves
    wcjo = np.ascontiguousarray(
        np.asarray(W, np.float32).reshape(C2, C1, 4).transpose(1, 2, 0))
    wts = np.ascontiguousarray(np.concatenate([wcjo, wcjo], axis=0))
    bias = np.ascontiguousarray(np.asarray(b, np.float32).reshape(C2, 1))
    return wts, bias


def make_weight_inputs(W, b):
    p = np.arange(128)
    c, s1 = p % 64, p // 64
    wt0 = np.ascontiguousarray(W[:, c * 4 + s1 * 2 + 0].T, dtype=np.float32)
    wt1 = np.ascontiguousarray(W[:, c * 4 + s1 * 2 + 1].T, dtype=np.float32)
    bias = np.ascontiguousarray(np.asarray(b, np.float32).reshape(C2, 1))
    return wt0, wt1, bias


def kernel(x, W, b):
    x = np.asarray(x, dtype=np.float32)
    W = np.asarray(W, dtype=np.float32)
    wts, bias = make_weight_inputs2(W, b)
    in_maps = [
        {
            "xs": np.ascontiguousarray(x[i * BPC:(i + 1) * BPC]),
            "wts": wts,
            "bias": bias,
        }
        for i in range(N_CORES)
    ]
    nc = build_nc2()
    res = run_bass_kernel_spmd(nc, in_maps, list(range(N_CORES)))
    return np.concatenate([res.results[i]["y"] for i in range(N_CORES)], axis=0)
